# revision 1
# baseline (speedup 1.0000x reference)
"""Bahdanau additive attention on 8 TRN2 NeuronCores -- Fourier/harmonic kernel.

Replaces the O(T*S*D) pointwise tanh (the baseline's ACT-engine wall at
~163us/core) with a separable harmonic expansion:

    tanh(z) ~= sum_{r=1..R} c_r sin(r*om0*z),   z = wq[t,d] + uh[s,d]
    sin(r*om0*(a+b)) = sin(r*om0*a)cos(r*om0*b) + cos(r*om0*a)sin(r*om0*b)

so align[t,s] = sum_r sum_d (c_r v_d sin_ra[t,d]) cos_rb[s,d] + (...) --
2R matmuls over d on the PE instead of T*S*D tanh evals.  The sin/cos
arrays live only on the small [T,D]+[S,D] sides:

  * ACT computes half-angle bases sin/cos(om0/2 * x) (args <= 2.7, inside
    the HW sin table's exact range |x|<3).
  * DVE bootstraps s~1 = sh*ch = sin(om0 x)/2 and c^1 = 2-4sh^2 =
    2cos(om0 x), then higher harmonics via the Chebyshev recurrence
    x_r = c^1 * x_{r-1} - x_{r-2} (2 fp16 tensor_tensor ops per output,
    2x DVE mode).  The s-chain runs at half scale and the c-chain at
    double scale so products s~a*c^b == sin_a*cos_b need no fixups.
  * The b-side (all four batches' uh, sum S_eff columns) is split
    column-wise between DVE and GpSimd(Pool) to run both engines.
  * v_d (and per-r c_r) fold into the a-side chain seeds / scaled copies,
    so the b-side needs no scaling at all.

Sharding, S_eff mask truncation, the additive-mask rank-1 matmuls, fused
softmax, and the output projection epilogue are inherited from the
baseline kernel; all matmul operands are fp16 (validated: end-to-end
maxrel ~2e-3 incl. fp16 recurrence rounding vs the 2e-2 gate).
"""
import numpy as np
from contextlib import ExitStack

import concourse.bass as bass
import concourse.bacc as bacc
import concourse.mybir as mybir
import concourse.tile as tile
from concourse.bass_utils import run_bass_kernel_spmd

F32 = mybir.dt.float32
F16 = mybir.dt.float16
SIN = mybir.ActivationFunctionType.Sin
EXP = mybir.ActivationFunctionType.Exp
IDENT = mybir.ActivationFunctionType.Identity
MUL = mybir.AluOpType.mult
SUB = mybir.AluOpType.subtract
ADD = mybir.AluOpType.add
F16np = np.float16

B, T, S, D, IN = 4, 512, 512, 256, 512
NC = 8
NJ = 2
TT = 128
SEG = 64

# tanh(z) ~= sum_r C[r-1] * sin(r*OM0*z), fitted on |z|<=8.85
R = 8
OM0 = 0.288272404
C = [1.130780854, 0.1794194439, 0.0871046907, 0.2588515218,
     -0.1505643306, 0.2580629394, -0.1491436225, 0.09975142414]
POOL_FRAC = 0.20   # fraction of b-side columns handled by GpSimd

_BUILT = {}
LAST_RESULT = None


def _bsplit(seffs):
    """Column split of the concatenated b-side [b0|b1|b2|b3] between the
    DVE-owned and Pool-owned tiles.  Pool gets batch3 plus a tail of batch2.
    Returns (WD, WP, rhs_map) with rhs_map[b] = [(which, off, w, col0)]."""
    total = sum(seffs)
    pt = int(round(POOL_FRAC * total))
    pw2 = min(seffs[2], max(0, pt - seffs[3]))
    wd = seffs[0] + seffs[1] + (seffs[2] - pw2)
    wp = pw2 + seffs[3]
    rhs = {
        0: [("d", 0, seffs[0], 0)],
        1: [("d", seffs[0], seffs[1], 0)],
        2: [("d", seffs[0] + seffs[1], seffs[2] - pw2, 0)]
           + ([("p", 0, pw2, seffs[2] - pw2)] if pw2 else []),
        3: [("p", pw2, seffs[3], 0)],
    }
    return wd, wp, rhs


def _build(seffs):
    nc = bacc.Bacc("TRN2", target_bir_lowering=False, debug=False,
                   enable_asserts=False, num_devices=NC)
    WD, WP, RHS = _bsplit(seffs)

    xT_d = nc.dram_tensor("xT", [NJ, 4, 128, TT], F16, kind="ExternalInput")
    memsT_d = nc.dram_tensor("memsT", [4, 2, 128, S], F16, kind="ExternalInput")
    memsL_d = nc.dram_tensor("memsL", [4, 128, 4, D], F16, kind="ExternalInput")
    maskseg_d = nc.dram_tensor("maskseg", [NJ, 2, 1, S], F16, kind="ExternalInput")
    indic_d = nc.dram_tensor("indic", [2, 1, 128], F16, kind="ExternalInput")
    ones_d = nc.dram_tensor("ones1", [1, 128], F16, kind="ExternalInput")
    boutw_d = nc.dram_tensor("boutw", [1, IN], F16, kind="ExternalInput")
    WqT_d = nc.dram_tensor("WqT", [4, 128, D], F16, kind="ExternalInput")
    WcT_d = nc.dram_tensor("WcT", [2, 128, D], F16, kind="ExternalInput")
    WoCT_d = nc.dram_tensor("WoCT", [128, 2, IN], F16, kind="ExternalInput")
    WoXT_d = nc.dram_tensor("WoXT", [128, 4, IN], F16, kind="ExternalInput")
    ident_d = nc.dram_tensor("ident", [128, 128], F16, kind="ExternalInput")
    VB_d = nc.dram_tensor("VB", [128, 2 * 2 * TT], F16, kind="ExternalInput")
    VB2_d = nc.dram_tensor("VB2", [128, 2 * 2 * TT], F16, kind="ExternalInput")
    CC_d = nc.dram_tensor("CC", [128, 2], F32, kind="ExternalInput")
    CR_d = nc.dram_tensor("CR", [128, 16], F32, kind="ExternalInput")

    attn_d = nc.dram_tensor("attn_outT", [NJ, 128, 4, TT], F32, kind="ExternalOutput")
    align_d = nc.dram_tensor("align_out", [NJ, 128, S], F16, kind="ExternalOutput")

    def seff(j, k):
        return seffs[2 * j + k]

    with tile.TileContext(nc) as tc, ExitStack() as ctx:
        const = ctx.enter_context(tc.tile_pool(name="const", bufs=1))
        pin = ctx.enter_context(tc.tile_pool(name="pin", bufs=1))
        pbase = ctx.enter_context(tc.tile_pool(name="pbase", bufs=1))
        pscr = ctx.enter_context(tc.tile_pool(name="pscr", bufs=1))
        pbd = ctx.enter_context(tc.tile_pool(name="pbd", bufs=6))
        pbp = ctx.enter_context(tc.tile_pool(name="pbp", bufs=6))
        pa = ctx.enter_context(tc.tile_pool(name="pa", bufs=5))
        pw = ctx.enter_context(tc.tile_pool(name="pw", bufs=2))
        pep = ctx.enter_context(tc.tile_pool(name="pep", bufs=NJ))
        psW = ctx.enter_context(tc.tile_pool(name="psW", bufs=1, space="PSUM"))
        psU = ctx.enter_context(tc.tile_pool(name="psU", bufs=1, space="PSUM"))
        psA = ctx.enter_context(tc.tile_pool(name="psA", bufs=1, space="PSUM"))
        psT = ctx.enter_context(tc.tile_pool(name="psT", bufs=2, space="PSUM"))
        psO = ctx.enter_context(tc.tile_pool(name="psO", bufs=1, space="PSUM"))

        def load(pool, shape, dt, src, tag, engine=None):
            t = pool.tile(shape, dt, tag=tag)
            (engine or nc.sync).dma_start(t[...], src)
            return t

        # ---- input DMAs: uh(b2,b3) chain is the longest pole, load first ----
        CCt = load(const, [128, 2], F32, CC_d.ap(), "CC")
        wcTc = [load(const, [128, D], F16, WcT_d.ap()[mc], f"wcTc{mc}")
                for mc in range(2)]
        mTs = {}
        for b in [2, 3]:
            mTs[b] = [load(pin, [128, S], F16, memsT_d.ap()[b][mc], f"mT{b}c{mc}")
                      for mc in range(2)]
        wqTc = [load(const, [128, D], F16, WqT_d.ap()[ic], f"wqTc{ic}")
                for ic in range(4)]
        xTc = [[load(pin, [128, TT], F16, xT_d.ap()[j][ic], f"xT{j}c{ic}")
                for ic in range(4)] for j in range(NJ)]
        for b in [0, 1]:
            mTs[b] = [load(pin, [128, S], F16, memsT_d.ap()[b][mc], f"mT{b}c{mc}")
                      for mc in range(2)]
        VB = load(const, [128, 2 * 2 * TT], F16, VB_d.ap(), "VB")
        VB2 = load(const, [128, 2 * 2 * TT], F16, VB2_d.ap(), "VB2")
        CRt = load(const, [128, 16], F32, CR_d.ap(), "CR", nc.scalar)
        masksegs = [[load(pin, [1, S], F16, maskseg_d.ap()[j][k], f"msk{j}{k}",
                          nc.scalar)
                     for k in range(2)] for j in range(NJ)]
        indics = [load(const, [1, 128], F16, indic_d.ap()[k], f"indic{k}",
                       nc.scalar)
                  for k in range(2)]
        ones1 = load(const, [1, 128], F16, ones_d.ap(), "ones1", nc.scalar)
        boutw = load(const, [1, IN], F16, boutw_d.ap(), "boutw", nc.scalar)
        woCT = load(const, [128, 2, IN], F16, WoCT_d.ap(), "woCT", nc.scalar)
        woXT = load(const, [128, 4, IN], F16, WoXT_d.ap(), "woXT", nc.scalar)
        ident = load(const, [128, 128], F16, ident_d.ap(), "ident", nc.scalar)
        memsLs = {}
        for b in range(4):
            nch = (seffs[b] + 127) // 128
            memsLs[b] = load(pin, [128, nch, D], F16,
                             memsL_d.ap()[b][:, :nch, :], f"memsL{b}", nc.scalar)

        # ---- phase 1: wq matmuls + a-side bases ----
        AW = 2 * NJ * TT   # flat a-side width: col = h*2TT + j*TT + t
        sh_a = pbase.tile([128, AW], F16, tag="sh_a")
        ch_a = pbase.tile([128, AW], F16, tag="ch_a")
        for j in range(NJ):
            wq_ps = psW.tile([128, 2, TT], F32, tag="wqc", name=f"wq{j}")
            for h in range(2):
                for ic in range(4):
                    nc.tensor.matmul(wq_ps[:, h, :],
                                     wqTc[ic][:, h * 128:(h + 1) * 128],
                                     xTc[j][ic][...],
                                     start=(ic == 0), stop=(ic == 3))
            for h in range(2):
                a0 = h * 2 * TT + j * TT
                nc.scalar.activation(sh_a[:, a0:a0 + TT], wq_ps[:, h, :],
                                     SIN, scale=CCt[:, 0:1])
                nc.scalar.activation(ch_a[:, a0:a0 + TT], wq_ps[:, h, :],
                                     SIN, scale=CCt[:, 0:1], bias=CCt[:, 1:2])

        # a-side bootstrap (DVE): c1d_a = 2cos(om0 a) unscaled coefficient,
        # chain seeds v-scaled (recurrence is linear, v_d commutes)
        t0a = pscr.tile([128, AW], F16, tag="t0a")
        nc.vector.tensor_tensor(t0a[...], sh_a[...], sh_a[...], MUL)
        c1dd_a = pbase.tile([128, 2 * AW], F16, tag="c1dd_a")
        nc.vector.tensor_scalar(c1dd_a[:, :AW], t0a[...], -4.0, 2.0, MUL, ADD)
        nc.vector.tensor_scalar(c1dd_a[:, AW:], t0a[...], -4.0, 2.0, MUL, ADD)
        s1h_a = pscr.tile([128, AW], F16, tag="s1h_a")
        nc.vector.tensor_tensor(s1h_a[...], sh_a[...], ch_a[...], MUL)
        ag1 = pbase.tile([128, 2 * AW], F16, tag="ag1")
        nc.vector.tensor_tensor(ag1[:, :AW], s1h_a[...], VB[...], MUL)
        nc.vector.tensor_tensor(ag1[:, AW:], c1dd_a[:, :AW], VB[...], MUL)
        a_g = {1: ag1}

        # ---- phase 2: uh matmuls + b-side bases (batches 2,3 first: Pool) ----
        sh_bd = pbase.tile([128, 2 * WD], F16, tag="sh_bd")
        sh_bp = pbase.tile([128, 2 * WP], F16, tag="sh_bp")
        ch_bd = pbase.tile([128, 2 * WD], F16, tag="ch_bd")
        ch_bp = pbase.tile([128, 2 * WP], F16, tag="ch_bp")
        sh_b = {"d": sh_bd, "p": sh_bp}
        ch_b = {"d": ch_bd, "p": ch_bp}
        for b in [2, 3, 0, 1]:
            sk = seffs[b]
            uh_ps = psU.tile([128, 2, sk], F32, tag="uh",
                             padded_shape=[128, 2, S], name=f"uh{b}")
            for h in range(2):
                for mc in range(2):
                    nc.tensor.matmul(uh_ps[:, h, :],
                                     wcTc[mc][:, h * 128:(h + 1) * 128],
                                     mTs[b][mc][:, :sk],
                                     start=(mc == 0), stop=(mc == 1))
            for (which, off, w, c0) in RHS[b]:
                W = WD if which == "d" else WP
                for h in range(2):
                    nc.scalar.activation(sh_b[which][:, h * W + off:
                                                     h * W + off + w],
                                         uh_ps[:, h, c0:c0 + w], SIN,
                                         scale=CCt[:, 0:1])
                    nc.scalar.activation(ch_b[which][:, h * W + off:
                                                     h * W + off + w],
                                         uh_ps[:, h, c0:c0 + w], SIN,
                                         scale=CCt[:, 0:1], bias=CCt[:, 1:2])

        # b-side bootstrap per engine tile; generation tiles hold [s|c]
        # merged along the free axis (col = kind*2W + h*W + off) so each
        # recurrence step is 2 wide ops instead of 4
        b_g, c1dd_b = {}, {}
        for which, W, eng in ([("d", WD, nc.vector)] +
                               ([("p", WP, nc.gpsimd)] if WP else [])):
            t0 = pscr.tile([128, 2 * W], F16, tag=f"t0{which}")
            eng.tensor_tensor(t0[...], sh_b[which][...], sh_b[which][...], MUL)
            c1dd = pbase.tile([128, 4 * W], F16, tag=f"c1dd{which}")
            nc.vector.tensor_scalar(c1dd[:, :2 * W], t0[...], -4.0, 2.0, MUL, ADD)
            nc.vector.tensor_scalar(c1dd[:, 2 * W:], t0[...], -4.0, 2.0, MUL, ADD)
            g1 = pbase.tile([128, 4 * W], F16, tag=f"g1{which}")
            eng.tensor_tensor(g1[:, :2 * W], sh_b[which][...], ch_b[which][...],
                              MUL)
            nc.vector.tensor_copy(g1[:, 2 * W:], c1dd[:, :2 * W])
            c1dd_b[which] = c1dd
            b_g[which] = {1: g1}

        # ---- phase 3: harmonic chains + align matmuls ----
        al0 = psA.tile([128, S], F32, tag="al0")
        al1 = psA.tile([128, S], F32, tag="al1")
        align_pss = [al0, al1]

        c2dd_b = {}

        def gen_b(which, r, eng):
            W = WD if which == "d" else WP
            gr = (pbd if which == "d" else pbp).tile(
                [128, 4 * W], F16, tag="bg" if which == "d" else "pg",
                name=f"bg_{which}{r}")
            gd = b_g[which]
            if r == 2:
                eng.tensor_tensor(gr[...], c1dd_b[which][...], gd[1][...], MUL)
                nc.vector.tensor_scalar_add(gr[:, 2 * W:], gr[:, 2 * W:], -2.0)
                c2dd = pbase.tile([128, 4 * W], F16, tag=f"c2dd{which}")
                eng.tensor_copy(c2dd[:, :2 * W], gr[:, 2 * W:])
                eng.tensor_copy(c2dd[:, 2 * W:], gr[:, 2 * W:])
                c2dd_b[which] = c2dd
            elif r == 3:
                eng.tensor_tensor(gr[...], c2dd_b[which][...], gd[1][...], MUL)
                eng.tensor_tensor(gr[:, :2 * W], gr[:, :2 * W],
                                  gd[1][:, :2 * W], ADD)
                eng.tensor_tensor(gr[:, 2 * W:], gr[:, 2 * W:],
                                  gd[1][:, 2 * W:], SUB)
            elif r % 2 == 1:
                eng.tensor_tensor(gr[...], c2dd_b[which][...], gd[r - 2][...],
                                  MUL)
                eng.tensor_tensor(gr[...], gr[...], gd[r - 4][...], SUB)
            else:
                m = r // 2
                eng.tensor_tensor(gr[:, :2 * W], gd[m][:, :2 * W],
                                  gd[m][:, 2 * W:], MUL)
                eng.tensor_tensor(gr[:, 2 * W:], gd[m][:, :2 * W],
                                  gd[m][:, :2 * W], MUL)
                nc.vector.tensor_scalar(gr[:, 2 * W:], gr[:, 2 * W:],
                                        -16.0, 2.0, MUL, ADD)
            b_g[which][r] = gr

        c2dd_a = [None]

        def gen_a(r):
            gr = pa.tile([128, 2 * AW], F16, tag="ag", name=f"ag{r}")
            if r == 2:
                nc.vector.tensor_tensor(gr[...], c1dd_a[...], a_g[1][...], MUL)
                nc.vector.tensor_tensor(gr[:, AW:], gr[:, AW:], VB2[...], SUB)
                # a-chain is v-scaled; coefficient 2cos(2w0 a) must be
                # unscaled: c2 = c1d^2 - 2
                c2 = pbase.tile([128, AW], F16, tag="c2a")
                nc.vector.tensor_tensor(c2[...], c1dd_a[:, :AW],
                                        c1dd_a[:, :AW], MUL)
                nc.vector.tensor_scalar_add(c2[...], c2[...], -2.0)
                c2dd = pbase.tile([128, 2 * AW], F16, tag="c2dd_a")
                nc.vector.tensor_copy(c2dd[:, :AW], c2[...])
                nc.vector.tensor_copy(c2dd[:, AW:], c2[...])
                c2dd_a[0] = c2dd
            elif r == 3:
                nc.vector.tensor_tensor(gr[...], c2dd_a[0][...], a_g[1][...],
                                        MUL)
                nc.vector.tensor_tensor(gr[:, :AW], gr[:, :AW],
                                        a_g[1][:, :AW], ADD)
                nc.vector.tensor_tensor(gr[:, AW:], gr[:, AW:],
                                        a_g[1][:, AW:], SUB)
            elif r % 2 == 1:
                nc.vector.tensor_tensor(gr[...], c2dd_a[0][...],
                                        a_g[r - 2][...], MUL)
                nc.vector.tensor_tensor(gr[...], gr[...], a_g[r - 4][...], SUB)
            else:
                # v-scaled squaring: s~'_2m = s~'_m * c^_m / v ... not scale
                # -clean for the a-side; instead use the unscaled-coefficient
                # stride-1 recurrence for evens too? No: use s~'_2m =
                # s~'_m * c^raw_m requires unscaled c^_m. Generate evens from
                # the stride-1 Chebyshev instead: c1dd * g_{r-1} - g_{r-2}.
                nc.vector.tensor_tensor(gr[...], c1dd_a[...], a_g[r - 1][...],
                                        MUL)
                nc.vector.tensor_tensor(gr[...], gr[...], a_g[r - 2][...], SUB)
            a_g[r] = gr

        assert R in (8, 9, 10)
        ORDER = list(range(1, R + 1))
        for r in ORDER:
            if r >= 2:
                if WP:
                    gen_b("p", r, nc.gpsimd)
                gen_a(r)
                gen_b("d", r, nc.vector)
            wsc = pw.tile([128, 2 * AW], F16, tag="wsc", name=f"wsc{r}")
            nc.vector.tensor_scalar_mul(wsc[...], a_g[r][...], float(C[r - 1]))
            for j in range(NJ):
                for k in range(2):
                    b = 2 * j + k
                    for pi, (which, off, w, c0) in enumerate(RHS[b]):
                        W = WD if which == "d" else WP
                        for h in range(2):
                            for kind in range(2):
                                # kind 0: sin_a x cos_b; kind 1: cos_a x sin_b
                                a0 = (kind * AW + h * 2 * TT + j * TT
                                      + SEG * k)
                                lhsT = wsc[:, a0:a0 + SEG]
                                b0c = (1 - kind) * 2 * W
                                rhs = b_g[which][r][
                                    :, b0c + h * W + off: b0c + h * W + off + w]
                                first = (r == 1 and h == 0 and kind == 0
                                         and pi == 0)
                                nc.tensor.matmul(
                                    align_pss[j][SEG * k:SEG * (k + 1),
                                                 c0:c0 + w],
                                    lhsT, rhs,
                                    start=first, stop=False,
                                    tile_position=(0, SEG * k),
                                    skip_group_check=True)

        def emit_epilogue(j):
            align_ps = align_pss[j]
            for k in range(2):
                nc.tensor.matmul(align_ps[...], indics[k][...],
                                 masksegs[j][k][...],
                                 start=False, stop=(k == 1),
                                 skip_group_check=True)

            av_e = pep.tile([128, S], F32, tag="av_e")
            ssum = pep.tile([128, 1], F32, tag="ssum")
            nc.scalar.activation(av_e[...], align_ps[...], EXP,
                                 accum_out=ssum[...])

            at_ps = psO.tile([128, 4 * TT], F32, tag="at_ps")
            for oc in range(4):
                for ic in range(4):
                    nc.tensor.matmul(at_ps[:, oc * TT:(oc + 1) * TT],
                                     woXT[:, ic, oc * 128:(oc + 1) * 128],
                                     xTc[j][ic][...],
                                     start=(oc == 0 and ic == 0), stop=False)
                nc.tensor.matmul(at_ps[:, oc * TT:(oc + 1) * TT],
                                 boutw[:, oc * 128:(oc + 1) * 128],
                                 ones1[...],
                                 start=False, stop=False,
                                 skip_group_check=True)

            rcp = pep.tile([128, 1], F32, tag="rcp")
            nc.vector.reciprocal(rcp[...], ssum[...])
            av16 = pep.tile([128, S], F16, tag="av16")
            nc.vector.tensor_scalar_mul(av16[...], av_e[...], rcp[...])

            avTs = []
            for sb in range(4):
                tp = psT.tile([128, 128], F16, tag="tp")
                nc.tensor.transpose(tp[...], av16[:, sb * 128:(sb + 1) * 128],
                                    ident[...])
                avT = pep.tile([128, TT], F16, tag=f"avT{sb}")
                nc.vector.tensor_copy(avT[...], tp[...])
                avTs.append(avT)

            nc.sync.dma_start(align_d.ap()[j], av16[...])

            c_ps = psW.tile([128, 2, TT], F32, tag="wqc", name=f"c_ps{j}")
            first = True
            c_bfs = []
            for mh in range(2):
                for k in range(2):
                    b = 2 * j + k
                    nch = (seffs[b] + 127) // 128
                    for sb in range(nch):
                        nc.tensor.matmul(
                            c_ps[:, mh, k * SEG:(k + 1) * SEG],
                            memsLs[b][:, sb, mh * 128:(mh + 1) * 128],
                            avTs[sb][:, k * SEG:(k + 1) * SEG],
                            start=first, stop=False,
                            skip_group_check=True)
                        first = False
                c_bf = pep.tile([128, TT], F16, tag=f"c_bf{mh}")
                nc.vector.tensor_copy(c_bf[...], c_ps[:, mh, :])
                c_bfs.append(c_bf)

            for oc in range(4):
                for mh in range(2):
                    nc.tensor.matmul(at_ps[:, oc * TT:(oc + 1) * TT],
                                     woCT[:, mh, oc * 128:(oc + 1) * 128],
                                     c_bfs[mh][...],
                                     start=False, stop=(oc == 3 and mh == 1))
                attn_sb = pep.tile([128, TT], F32, tag=f"attn_sb{oc}")
                nc.vector.tensor_copy(attn_sb[...],
                                      at_ps[:, oc * TT:(oc + 1) * TT])
                nc.sync.dma_start(attn_d.ap()[j][:, oc, :], attn_sb[...])

        emit_epilogue(0)
        emit_epilogue(1)

    nc.compile()
    return nc


def _to_chunks(a, nch):
    return np.ascontiguousarray(a.reshape(nch, 128, a.shape[-1]))


def _to_pcf(a, nch):
    return np.ascontiguousarray(a.reshape(nch, 128, a.shape[-1]).transpose(1, 0, 2))


def _prep_inputs(inputs, mems, mem_masks, Wq, Wc, bc, v, Wout, bout):
    x = np.ascontiguousarray(np.asarray(inputs, dtype=np.float32))
    mems = np.ascontiguousarray(np.asarray(mems, dtype=np.float32))
    L = np.asarray(mem_masks).astype(np.int64)
    Wq = np.asarray(Wq, dtype=np.float32)
    Wc = np.asarray(Wc, dtype=np.float32)
    bc = np.asarray(bc, dtype=np.float32)
    v = np.asarray(v, dtype=np.float32)
    Wout = np.asarray(Wout, dtype=np.float32)
    bout = np.asarray(bout, dtype=np.float32)
    assert np.all(bc == 0.0), "kernel folds bc into ACT bias cols; bc!=0 unsupported"

    seffs = tuple(int(min(max(((int(l) + 1) // 2) * 2, 2), S)) for l in L)

    WqT = _to_chunks(np.ascontiguousarray(Wq.T), 4).astype(F16np)
    WcT = _to_chunks(np.ascontiguousarray(Wc.T), 2).astype(F16np)
    WoCT = _to_pcf(np.ascontiguousarray(Wout[:, :D].T), 2).astype(F16np)
    WoXT = _to_pcf(np.ascontiguousarray(Wout[:, D:].T), 4).astype(F16np)
    ident = np.eye(128, dtype=np.float32).astype(F16np)
    indic = np.zeros((2, 1, 128), np.float32)
    indic[0, 0, :SEG] = 1.0
    indic[1, 0, SEG:] = 1.0

    VB = np.zeros((128, 2, 2 * TT), np.float32)
    for h in range(2):
        VB[:, h, :] = v[h * 128:(h + 1) * 128][:, None]
    VB = VB.reshape(128, 2 * 2 * TT)
    CC = np.zeros((128, 2), np.float32)
    CC[:, 0] = OM0 / 2
    CC[:, 1] = np.pi / 2
    CR = np.zeros((128, 16), np.float32)
    for r in range(R):
        CR[:, r] = C[r]

    shared = dict(WqT=WqT, WcT=WcT, WoCT=WoCT, WoXT=WoXT, ident=ident,
                  indic=indic.astype(F16np),
                  boutw=bout.reshape(1, IN).astype(F16np),
                  ones1=np.ones((1, 128), np.float32).astype(F16np),
                  VB=VB.astype(F16np), VB2=(2 * VB).astype(F16np),
                  CC=CC, CR=CR)

    memsT = np.zeros((4, 2, 128, S), np.float32)
    memsL = np.zeros((4, 128, 4, D), np.float32)
    for b in range(4):
        memsT[b] = _to_chunks(np.ascontiguousarray(mems[b].T), 2)
        memsL[b] = _to_pcf(mems[b], 4)

    in_maps = []
    for core in range(NC):
        r0 = core * SEG
        xT = np.zeros((NJ, 4, 128, TT), np.float32)
        maskseg = np.zeros((NJ, 2, 1, S), np.float32)
        for j in range(NJ):
            xrows = np.concatenate(
                [x[2 * j, r0:r0 + SEG, :], x[2 * j + 1, r0:r0 + SEG, :]], 0)
            xT[j] = _to_chunks(np.ascontiguousarray(xrows.T), 4)
            for k in range(2):
                b = 2 * j + k
                maskseg[j, k, 0, :] = np.where(np.arange(S) < int(L[b]),
                                               0.0, -30.0)
        m = dict(shared)
        m["xT"] = np.ascontiguousarray(xT).astype(F16np)
        m["memsT"] = np.ascontiguousarray(memsT).astype(F16np)
        m["memsL"] = np.ascontiguousarray(memsL).astype(F16np)
        m["maskseg"] = np.ascontiguousarray(maskseg).astype(F16np)
        in_maps.append(m)
    return in_maps, seffs


def kernel(**inputs):
    global LAST_RESULT
    in_maps, seffs = _prep_inputs(**inputs)
    if seffs not in _BUILT:
        _BUILT[seffs] = _build(seffs)
    res = run_bass_kernel_spmd(_BUILT[seffs], in_maps, core_ids=list(range(NC)))
    LAST_RESULT = res

    attn_h = np.zeros((B, T, IN), np.float32)
    align_v = np.zeros((B, T, S), np.float32)
    for core in range(NC):
        r0 = core * SEG
        for j in range(NJ):
            at = res.results[core]["attn_outT"][j]
            blk = np.transpose(at, (2, 1, 0)).reshape(TT, IN)
            al = res.results[core]["align_out"][j].astype(np.float32)
            for k in range(2):
                b = 2 * j + k
                attn_h[b, r0:r0 + SEG, :] = blk[k * SEG:(k + 1) * SEG]
                align_v[b, r0:r0 + SEG, :] = al[k * SEG:(k + 1) * SEG]
    return attn_h, align_v



# revision 5
# speedup vs baseline: 2.0197x; 2.0197x over previous
"""Bahdanau additive attention on 8 TRN2 NeuronCores -- harmonic kernel v2.

Same Fourier/harmonic math as v1 (tanh(z) ~= sum_r c_r sin(r*om0*z), angle
-addition turns the [T,S,D] pointwise tanh into 2R matmuls), but resharded:

  v1: 8 cores = 8 t-slices x ALL 4 batches -> every core rebuilt the b-side
      harmonic basis for all ~1522 active mem columns (the 112us DVE wall).
  v2: 8 cores = 4 batches x 2 t-halves -> each core owns ONE batch (b-side
      chain is 512 cols, ~3x less DVE work) and 256 t-rows, so the align
      matmuls run at M=128 (full PE stationary dim) instead of M=64.

Other changes vs v1: uniform S=512 with additive -30 masking (no data-
dependent shapes / rebuilds), v broadcast folded into per-partition
tensor_scalar ops (no [128,1024] VB tensors or their DMAs), per-r C[r]
scaling on the ACT engine, inputs packed into ~10 DMA descriptors spread
over 3 issue queues, exp table pre-warmed during the chain.
"""
import numpy as np
from contextlib import ExitStack

import concourse.bass as bass
import concourse.bacc as bacc
import concourse.mybir as mybir
import concourse.tile as tile
from concourse.bass_utils import run_bass_kernel_spmd

F32 = mybir.dt.float32
F16 = mybir.dt.float16
SIN = mybir.ActivationFunctionType.Sin
EXP = mybir.ActivationFunctionType.Exp
IDENT = mybir.ActivationFunctionType.Identity
MUL = mybir.AluOpType.mult
SUB = mybir.AluOpType.subtract
ADD = mybir.AluOpType.add
F16np = np.float16

B, T, S, D, IN = 4, 512, 512, 256, 512
NC = 8
TL = 256            # t rows per core (2 chunks of 128)
AW = 512            # a-side cols: 2 d-halves x TL
WD, WP = 368, 144   # b-side s-column split: DVE | GpSimd

R = 8
OM0 = 0.288272404
C = [1.130780854, 0.1794194439, 0.0871046907, 0.2588515218,
     -0.1505643306, 0.2580629394, -0.1491436225, 0.09975142414]

_BUILT = [None]
LAST_RESULT = None


def _build():
    nc = bacc.Bacc("TRN2", target_bir_lowering=False, debug=False,
                   enable_asserts=False, num_devices=NC)

    xT_d = nc.dram_tensor("xT", [128, 4, TL], F16, kind="ExternalInput")
    memsT_d = nc.dram_tensor("memsT", [128, 2, S], F16, kind="ExternalInput")
    memsL_d = nc.dram_tensor("memsL", [128, 4, D], F16, kind="ExternalInput")
    WqT_d = nc.dram_tensor("WqT", [128, 4, D], F16, kind="ExternalInput")
    WcT_d = nc.dram_tensor("WcT", [128, 2, D], F16, kind="ExternalInput")
    WoCT_d = nc.dram_tensor("WoCT", [128, 2, IN], F16, kind="ExternalInput")
    WoXT_d = nc.dram_tensor("WoXT", [128, 4, IN], F16, kind="ExternalInput")
    ident_d = nc.dram_tensor("ident", [128, 128], F16, kind="ExternalInput")
    CCV_d = nc.dram_tensor("CCV", [128, 6], F32, kind="ExternalInput")
    # [mask 512 | bout 512 | ones 256]
    PK1_d = nc.dram_tensor("PK1", [1, S + IN + TL], F16, kind="ExternalInput")

    attn_d = nc.dram_tensor("attn_outT", [128, 4, TL], F32,
                            kind="ExternalOutput")
    align_d = nc.dram_tensor("align_out", [2, 128, S], F16,
                             kind="ExternalOutput")

    with tile.TileContext(nc) as tc, ExitStack() as ctx:
        const = ctx.enter_context(tc.tile_pool(name="const", bufs=1))
        pbase = ctx.enter_context(tc.tile_pool(name="pbase", bufs=1))
        pscr = ctx.enter_context(tc.tile_pool(name="pscr", bufs=1))
        pbd = ctx.enter_context(tc.tile_pool(name="pbd", bufs=6))
        pbp = ctx.enter_context(tc.tile_pool(name="pbp", bufs=6))
        pa = ctx.enter_context(tc.tile_pool(name="pa", bufs=5))
        pw = ctx.enter_context(tc.tile_pool(name="pw", bufs=2))
        pep = ctx.enter_context(tc.tile_pool(name="pep", bufs=2))
        psW = ctx.enter_context(tc.tile_pool(name="psW", bufs=1, space="PSUM"))
        psU = ctx.enter_context(tc.tile_pool(name="psU", bufs=1, space="PSUM"))
        psA = ctx.enter_context(tc.tile_pool(name="psA", bufs=1, space="PSUM"))
        psT = ctx.enter_context(tc.tile_pool(name="psT", bufs=1, space="PSUM"))
        psO = ctx.enter_context(tc.tile_pool(name="psO", bufs=1, space="PSUM"))

        def load(pool, shape, dt, src, tag, engine):
            t = pool.tile(shape, dt, tag=tag)
            engine.dma_start(t[...], src)
            return t

        # ---- input DMAs over three issue queues ----
        # sync: b-side critical path; scalar: a-side (idle until uh done);
        # gpsimd: epilogue-only tensors (pool compute starts late)
        CCV = load(const, [128, 6], F32, CCV_d.ap(), "CCV", nc.sync)
        wcT = load(const, [128, 2, D], F16, WcT_d.ap(), "wcT", nc.sync)
        memsT = load(const, [128, 2, S], F16, memsT_d.ap(), "memsT", nc.sync)
        wqT = load(const, [128, 4, D], F16, WqT_d.ap(), "wqT", nc.scalar)
        xt = load(const, [128, 4, TL], F16, xT_d.ap(), "xt", nc.scalar)
        PK1 = load(const, [1, S + IN + TL], F16, PK1_d.ap(), "PK1", nc.gpsimd)
        woCT = load(const, [128, 2, IN], F16, WoCT_d.ap(), "woCT", nc.gpsimd)
        woXT = load(const, [128, 4, IN], F16, WoXT_d.ap(), "woXT", nc.gpsimd)
        memsL = load(const, [128, 4, D], F16, memsL_d.ap(), "memsL", nc.gpsimd)
        ident = load(const, [128, 128], F16, ident_d.ap(), "ident", nc.gpsimd)
        maskseg = PK1[:, 0:S]
        boutw = PK1[:, S:S + IN]
        ones = PK1[:, S + IN:]

        # ---- uh = mems @ Wc^T -> b-side half-angle seeds ----
        uh_ps = psU.tile([128, 2, S], F32, tag="uh")
        for h in range(2):
            for mc in range(2):
                nc.tensor.matmul(uh_ps[:, h, :],
                                 wcT[:, mc, h * 128:(h + 1) * 128],
                                 memsT[:, mc, :],
                                 start=(mc == 0), stop=(mc == 1))
        sh_bd = pbase.tile([128, 2 * WD], F16, tag="sh_bd")
        ch_bd = pbase.tile([128, 2 * WD], F16, tag="ch_bd")
        sh_bp = pbase.tile([128, 2 * WP], F16, tag="sh_bp")
        ch_bp = pbase.tile([128, 2 * WP], F16, tag="ch_bp")
        # pool-owned columns seeded first so GpSimd can start ASAP
        for h in range(2):
            nc.scalar.activation(sh_bp[:, h * WP:(h + 1) * WP],
                                 uh_ps[:, h, WD:], SIN, scale=CCV[:, 0:1])
            nc.scalar.activation(ch_bp[:, h * WP:(h + 1) * WP],
                                 uh_ps[:, h, WD:], SIN, scale=CCV[:, 0:1],
                                 bias=CCV[:, 1:2])
        for h in range(2):
            nc.scalar.activation(sh_bd[:, h * WD:(h + 1) * WD],
                                 uh_ps[:, h, :WD], SIN, scale=CCV[:, 0:1])
            nc.scalar.activation(ch_bd[:, h * WD:(h + 1) * WD],
                                 uh_ps[:, h, :WD], SIN, scale=CCV[:, 0:1],
                                 bias=CCV[:, 1:2])

        # ---- wq = x @ Wq^T -> a-side seeds ----
        wq_ps = psW.tile([128, 2, D], F32, tag="wq", name="wq")
        for h in range(2):
            for ic in range(4):
                nc.tensor.matmul(wq_ps[:, h, :TL],
                                 wqT[:, ic, h * 128:(h + 1) * 128],
                                 xt[:, ic, :],
                                 start=(ic == 0), stop=(ic == 3))
        sh_a = pbase.tile([128, AW], F16, tag="sh_a")
        ch_a = pbase.tile([128, AW], F16, tag="ch_a")
        for h in range(2):
            nc.scalar.activation(sh_a[:, h * TL:(h + 1) * TL],
                                 wq_ps[:, h, :TL], SIN, scale=CCV[:, 0:1])
            nc.scalar.activation(ch_a[:, h * TL:(h + 1) * TL],
                                 wq_ps[:, h, :TL], SIN, scale=CCV[:, 0:1],
                                 bias=CCV[:, 1:2])
        # pre-warm the exp table set during the chain phase
        prew = pscr.tile([128, 1], F32, tag="prew")
        nc.scalar.activation(prew[...], CCV[:, 0:1], EXP)

        # ---- output projection: x part + bias can accumulate early ----
        # NOTE on PSUM start bits: a start=True issued while the same bank's
        # accumulation bracket is open WIPES the open accumulation (verified
        # on HW) — so exactly one start per bank: the first matmul touching
        # it. at_ps spans 2 banks (oc0/1 and oc2/3).
        at_ps = psO.tile([128, 4, TL], F32, tag="at")
        for oc in range(4):
            for ic in range(4):
                nc.tensor.matmul(at_ps[:, oc, :],
                                 woXT[:, ic, oc * 128:(oc + 1) * 128],
                                 xt[:, ic, :],
                                 start=(ic == 0 and oc % 2 == 0), stop=False)
            nc.tensor.matmul(at_ps[:, oc, :],
                             boutw[:, oc * 128:(oc + 1) * 128],
                             ones[...],
                             start=False, stop=False, skip_group_check=True)

        # ---- bootstrap: s~1 = sin(om0 z)/2, c^1 = 2 cos(om0 z) ----
        # b-side, pool columns first
        t0p = pscr.tile([128, 2 * WP], F16, tag="t0p")
        nc.gpsimd.tensor_tensor(t0p[...], sh_bp[...], sh_bp[...], MUL)
        c1dd_p = pbase.tile([128, 4 * WP], F16, tag="c1dd_p")
        nc.vector.tensor_scalar(c1dd_p[:, :2 * WP], t0p[...], -4.0, 2.0,
                                MUL, ADD)
        nc.vector.tensor_scalar(c1dd_p[:, 2 * WP:], t0p[...], -4.0, 2.0,
                                MUL, ADD)
        g1p = pbase.tile([128, 4 * WP], F16, tag="g1p")
        nc.gpsimd.tensor_tensor(g1p[:, :2 * WP], sh_bp[...], ch_bp[...], MUL)
        nc.vector.tensor_copy(g1p[:, 2 * WP:], c1dd_p[:, :2 * WP])

        # a-side (v-scaled chain; v folded via per-partition tensor_scalar)
        t0a = pscr.tile([128, AW], F16, tag="t0a")
        nc.vector.tensor_tensor(t0a[...], sh_a[...], sh_a[...], MUL)
        c1dd_a = pbase.tile([128, 2 * AW], F16, tag="c1dd_a")
        nc.vector.tensor_scalar(c1dd_a[:, :AW], t0a[...], -4.0, 2.0, MUL, ADD)
        nc.vector.tensor_scalar(c1dd_a[:, AW:], t0a[...], -4.0, 2.0, MUL, ADD)
        s1h_a = pscr.tile([128, AW], F16, tag="s1h_a")
        nc.vector.tensor_tensor(s1h_a[...], sh_a[...], ch_a[...], MUL)
        ag1 = pbase.tile([128, 2 * AW], F16, tag="ag1")
        for h in range(2):
            vcol = CCV[:, 2 + h:3 + h]
            nc.vector.tensor_scalar_mul(ag1[:, h * TL:(h + 1) * TL],
                                        s1h_a[:, h * TL:(h + 1) * TL], vcol)
            nc.vector.tensor_scalar_mul(ag1[:, AW + h * TL:AW + (h + 1) * TL],
                                        c1dd_a[:, h * TL:(h + 1) * TL], vcol)
        a_g = {1: ag1}

        # b-side, DVE columns
        t0d = pscr.tile([128, 2 * WD], F16, tag="t0d")
        nc.vector.tensor_tensor(t0d[...], sh_bd[...], sh_bd[...], MUL)
        c1dd_d = pbase.tile([128, 4 * WD], F16, tag="c1dd_d")
        nc.vector.tensor_scalar(c1dd_d[:, :2 * WD], t0d[...], -4.0, 2.0,
                                MUL, ADD)
        nc.vector.tensor_scalar(c1dd_d[:, 2 * WD:], t0d[...], -4.0, 2.0,
                                MUL, ADD)
        g1d = pbase.tile([128, 4 * WD], F16, tag="g1d")
        nc.vector.tensor_tensor(g1d[:, :2 * WD], sh_bd[...], ch_bd[...], MUL)
        nc.vector.tensor_copy(g1d[:, 2 * WD:], c1dd_d[:, :2 * WD])

        b_g = {"d": {1: g1d}, "p": {1: g1p}}
        c1dd_b = {"d": c1dd_d, "p": c1dd_p}
        c2dd_b = {}

        al = [psA.tile([128, S], F32, tag=f"al{chnk}", name=f"al{chnk}")
              for chnk in range(2)]

        def gen_b(which, r, eng):
            W = WD if which == "d" else WP
            gr = (pbd if which == "d" else pbp).tile(
                [128, 4 * W], F16, tag="bg" if which == "d" else "pg",
                name=f"bg_{which}{r}")
            gd = b_g[which]
            if r == 2:
                eng.tensor_tensor(gr[...], c1dd_b[which][...], gd[1][...], MUL)
                nc.vector.tensor_scalar_add(gr[:, 2 * W:], gr[:, 2 * W:], -2.0)
                c2dd = pbase.tile([128, 4 * W], F16, tag=f"c2dd{which}")
                eng.tensor_copy(c2dd[:, :2 * W], gr[:, 2 * W:])
                eng.tensor_copy(c2dd[:, 2 * W:], gr[:, 2 * W:])
                c2dd_b[which] = c2dd
            elif r == 3:
                eng.tensor_tensor(gr[...], c2dd_b[which][...], gd[1][...], MUL)
                eng.tensor_tensor(gr[:, :2 * W], gr[:, :2 * W],
                                  gd[1][:, :2 * W], ADD)
                eng.tensor_tensor(gr[:, 2 * W:], gr[:, 2 * W:],
                                  gd[1][:, 2 * W:], SUB)
            elif r % 2 == 1:
                eng.tensor_tensor(gr[...], c2dd_b[which][...], gd[r - 2][...],
                                  MUL)
                eng.tensor_tensor(gr[...], gr[...], gd[r - 4][...], SUB)
            else:
                m = r // 2
                eng.tensor_tensor(gr[:, :2 * W], gd[m][:, :2 * W],
                                  gd[m][:, 2 * W:], MUL)
                eng.tensor_tensor(gr[:, 2 * W:], gd[m][:, :2 * W],
                                  gd[m][:, :2 * W], MUL)
                nc.vector.tensor_scalar(gr[:, 2 * W:], gr[:, 2 * W:],
                                        -16.0, 2.0, MUL, ADD)
            b_g[which][r] = gr

        c2dd_a = [None]

        def gen_a(r):
            gr = pa.tile([128, 2 * AW], F16, tag="ag", name=f"ag{r}")
            if r == 2:
                nc.vector.tensor_tensor(gr[...], c1dd_a[...], a_g[1][...], MUL)
                for h in range(2):
                    nc.vector.tensor_scalar(
                        gr[:, AW + h * TL:AW + (h + 1) * TL],
                        gr[:, AW + h * TL:AW + (h + 1) * TL],
                        CCV[:, 4 + h:5 + h], None, SUB)
                c2 = pbase.tile([128, AW], F16, tag="c2a")
                nc.vector.tensor_tensor(c2[...], c1dd_a[:, :AW],
                                        c1dd_a[:, :AW], MUL)
                nc.vector.tensor_scalar_add(c2[...], c2[...], -2.0)
                c2dd = pbase.tile([128, 2 * AW], F16, tag="c2dd_a")
                nc.vector.tensor_copy(c2dd[:, :AW], c2[...])
                nc.vector.tensor_copy(c2dd[:, AW:], c2[...])
                c2dd_a[0] = c2dd
            elif r == 3:
                nc.vector.tensor_tensor(gr[...], c2dd_a[0][...], a_g[1][...],
                                        MUL)
                nc.vector.tensor_tensor(gr[:, :AW], gr[:, :AW],
                                        a_g[1][:, :AW], ADD)
                nc.vector.tensor_tensor(gr[:, AW:], gr[:, AW:],
                                        a_g[1][:, AW:], SUB)
            elif r % 2 == 1:
                nc.vector.tensor_tensor(gr[...], c2dd_a[0][...],
                                        a_g[r - 2][...], MUL)
                nc.vector.tensor_tensor(gr[...], gr[...], a_g[r - 4][...], SUB)
            else:
                nc.vector.tensor_tensor(gr[...], c1dd_a[...], a_g[r - 1][...],
                                        MUL)
                nc.vector.tensor_tensor(gr[...], gr[...], a_g[r - 2][...], SUB)
            a_g[r] = gr

        # ---- harmonic chains + align matmuls ----
        for r in range(1, R + 1):
            if r >= 2:
                gen_b("p", r, nc.gpsimd)
                gen_a(r)
                gen_b("d", r, nc.vector)
            wsc = pw.tile([128, 2 * AW], F16, tag="wsc", name=f"wsc{r}")
            nc.scalar.activation(wsc[...], a_g[r][...], IDENT,
                                 scale=float(C[r - 1]))
            for chnk in range(2):
                for h in range(2):
                    for kind in range(2):
                        lhsT = wsc[:, kind * AW + h * TL + chnk * 128:
                                   kind * AW + h * TL + chnk * 128 + 128]
                        for which, W, c0 in (("d", WD, 0), ("p", WP, WD)):
                            b0c = (1 - kind) * 2 * W
                            rhs = b_g[which][r][:, b0c + h * W:
                                                b0c + h * W + W]
                            nc.tensor.matmul(
                                al[chnk][:, c0:c0 + W], lhsT, rhs,
                                start=(r == 1 and h == 0 and kind == 0
                                       and which == "d"),
                                stop=False, skip_group_check=True)

        # ---- per-chunk epilogue: mask, softmax, c, output projection ----
        def epi_mask(chnk):
            nc.tensor.matmul(al[chnk][...], ones[:, :128], maskseg[...],
                             start=False, stop=True, skip_group_check=True)

        def epi_softmax(chnk):
            av_e = pep.tile([128, S], F32, tag="av_e", name=f"av_e{chnk}")
            ssum = pep.tile([128, 1], F32, tag="ssum", name=f"ssum{chnk}")
            nc.scalar.activation(av_e[...], al[chnk][...], EXP,
                                 accum_out=ssum[...])
            rcp = pep.tile([128, 1], F32, tag="rcp", name=f"rcp{chnk}")
            nc.vector.reciprocal(rcp[...], ssum[...])
            av16 = pep.tile([128, S], F16, tag="av16", name=f"av16{chnk}")
            nc.vector.tensor_scalar_mul(av16[...], av_e[...], rcp[...])
            nc.sync.dma_start(align_d.ap()[chnk], av16[...])
            return av16

        def epi_ctx(chnk, av16):
            avTs = []
            for sb in range(4):
                tp = psT.tile([128, 128], F16, tag="tp", name=f"tp{chnk}{sb}")
                nc.tensor.transpose(tp[...], av16[:, sb * 128:(sb + 1) * 128],
                                    ident[...])
                avT = pep.tile([128, 128], F16, tag=f"avT{sb}",
                               name=f"avT{chnk}{sb}")
                nc.vector.tensor_copy(avT[...], tp[...])
                avTs.append(avT)
            c_ps = psW.tile([128, 2, D], F32, tag="wq", name=f"c_ps{chnk}")
            for mh in range(2):
                for sb in range(4):
                    nc.tensor.matmul(c_ps[:, mh, :128],
                                     memsL[:, sb, mh * 128:(mh + 1) * 128],
                                     avTs[sb][...],
                                     start=(sb == 0), stop=(sb == 3),
                                     skip_group_check=True)
            c_bfs = []
            for mh in range(2):
                c_bf = pep.tile([128, 128], F16, tag=f"c_bf{mh}",
                                name=f"c_bf{chnk}{mh}")
                nc.vector.tensor_copy(c_bf[...], c_ps[:, mh, :128])
                c_bfs.append(c_bf)
            return c_bfs

        def epi_out(chnk, c_bfs, last):
            for oc in range(4):
                for mh in range(2):
                    nc.tensor.matmul(
                        at_ps[:, oc, chnk * 128:(chnk + 1) * 128],
                        woCT[:, mh, oc * 128:(oc + 1) * 128],
                        c_bfs[mh][...],
                        start=False,
                        stop=(last and oc == 3 and mh == 1),
                        skip_group_check=True)

        epi_mask(0)
        epi_mask(1)
        av0 = epi_softmax(0)
        av1 = epi_softmax(1)
        cb0 = epi_ctx(0, av0)
        epi_out(0, cb0, last=False)
        cb1 = epi_ctx(1, av1)
        epi_out(1, cb1, last=True)

        at_bf = pep.tile([128, 4, TL], F32, tag="at_bf")
        nc.vector.tensor_copy(at_bf[...], at_ps[...])
        nc.sync.dma_start(attn_d.ap(), at_bf[...])

    nc.compile()
    return nc


def _pcf(a, nch):
    """[nch*128, W] -> [128, nch, W] partition-major."""
    return np.ascontiguousarray(
        a.reshape(nch, 128, a.shape[-1]).transpose(1, 0, 2))


def _prep_inputs(inputs, mems, mem_masks, Wq, Wc, bc, v, Wout, bout):
    x = np.asarray(inputs, dtype=np.float32)
    mems = np.asarray(mems, dtype=np.float32)
    L = np.asarray(mem_masks).astype(np.int64)
    Wq = np.asarray(Wq, dtype=np.float32)
    Wc = np.asarray(Wc, dtype=np.float32)
    bc = np.asarray(bc, dtype=np.float32)
    v = np.asarray(v, dtype=np.float32)
    Wout = np.asarray(Wout, dtype=np.float32)
    bout = np.asarray(bout, dtype=np.float32)
    assert np.all(bc == 0.0), "kernel folds bc into ACT bias; bc!=0 unsupported"

    WqT = _pcf(np.ascontiguousarray(Wq.T), 4).astype(F16np)
    WcT = _pcf(np.ascontiguousarray(Wc.T), 2).astype(F16np)
    WoCT = _pcf(np.ascontiguousarray(Wout[:, :D].T), 2).astype(F16np)
    WoXT = _pcf(np.ascontiguousarray(Wout[:, D:].T), 4).astype(F16np)
    ident = np.eye(128, dtype=np.float32).astype(F16np)
    CCV = np.zeros((128, 6), np.float32)
    CCV[:, 0] = OM0 / 2
    CCV[:, 1] = np.pi / 2
    CCV[:, 2] = v[:128]
    CCV[:, 3] = v[128:]
    CCV[:, 4] = 2 * v[:128]
    CCV[:, 5] = 2 * v[128:]

    shared = dict(WqT=WqT, WcT=WcT, WoCT=WoCT, WoXT=WoXT, ident=ident,
                  CCV=CCV)

    in_maps = []
    for core in range(NC):
        b, th = core // 2, core % 2
        t0 = th * TL
        xT = _pcf(np.ascontiguousarray(x[b, t0:t0 + TL, :].T), 4)
        memsT = _pcf(np.ascontiguousarray(mems[b].T), 2)
        memsL = _pcf(mems[b], 4)
        PK1 = np.zeros((1, S + IN + TL), np.float32)
        PK1[0, :S] = np.where(np.arange(S) < int(L[b]), 0.0, -30.0)
        PK1[0, S:S + IN] = bout
        PK1[0, S + IN:] = 1.0
        m = dict(shared)
        m["xT"] = xT.astype(F16np)
        m["memsT"] = memsT.astype(F16np)
        m["memsL"] = memsL.astype(F16np)
        m["PK1"] = PK1.astype(F16np)
        in_maps.append(m)
    return in_maps


def kernel(**inputs):
    global LAST_RESULT
    in_maps = _prep_inputs(**inputs)
    if _BUILT[0] is None:
        _BUILT[0] = _build()
    res = run_bass_kernel_spmd(_BUILT[0], in_maps, core_ids=list(range(NC)))
    LAST_RESULT = res

    attn_h = np.zeros((B, T, IN), np.float32)
    align_v = np.zeros((B, T, S), np.float32)
    for core in range(NC):
        b, th = core // 2, core % 2
        t0 = th * TL
        at = res.results[core]["attn_outT"]
        attn_h[b, t0:t0 + TL, :] = np.transpose(at, (2, 1, 0)).reshape(TL, IN)
        al = res.results[core]["align_out"].astype(np.float32)
        align_v[b, t0:t0 + TL, :] = al.reshape(TL, S)
    return attn_h, align_v
